# revision 29
# baseline (speedup 1.0000x reference)
"""Multi-head attention (QKV proj + rotary + softmax attention + out proj)
for Trainium2, sharded over 8 NeuronCores.

Problem: x[2,2048,1024], 16 heads x dh=64, rotary embedding, softmax
attention, output projection + bias.

Sharding: batch x head-group. Core c handles batch c//4 and the 4 heads
[4*(c%4), 4*(c%4)+4). Each core computes its QKV slice, rotary, attention,
and a partial output projection; the host sums the 4 partial projections
per batch and adds the bias.

Device-side design (per core, everything in "transposed" layout, all
matmul operands bf16/fp16 so every matmul streams at 1 col/cycle and
DMA+SBUF traffic is halved):
  - DMAs are emitted fine-grained (64KB-256KB) in consumption order so
    the first matmul can start ~3us in instead of waiting for all
    constants; all four xT n-tiles are persistent in SBUF.
  - qkvT e-chunks = W @ x^T accumulated over 8 d-chunks, in two 256-col
    half-units so fill work can be interleaved at ~850ns granularity.
  - rotary on the fp32 psum via DVE (+ the sin-multiply on the otherwise
    idle gpsimd engine): q*cos + pairswap(q*sin_pre), dh interleaved
    ([0,32,1,33,...]) so rotate_half is an adjacent-lane stream_shuffle.
    Outputs bf16.
  - dots: scoresT[j,n] = krotT-slice @ qrotT, two heads packed in the PE
    array via tile_position row-tiling (K=64 each). Per-(jl,h) psum
    tiles [128,512] with 4 psum banks of buffering so the PE never
    blocks on the exp pipeline.
  - softmax without max-subtraction (logits are O(+-6)): ACT exp per
    [128,512] psum tile, output fp16.
  - AV: lhsT = [v | ones] (M=65, fp16) so row 64 accumulates the softmax
    denominators for free; fp32 psum accumulation over the 16 j-tiles.
    AV emission is software-pipelined one j-batch behind the dots/exp so
    the in-order PE queue never waits on ACT.
  - normalize: reciprocal_approx_fast of the sums row,
    partition-broadcast via a DRAM round-trip DMA (K=1 ones-matmul on
    the last block to keep the tail off the DMA latency), one DVE
    multiply -> aoT (bf16).
  - output proj: y[n,d] accumulated over the two head-pair e-chunks,
    emitted per-128-row units threaded through later rounds; the last
    block's pair-0 partial goes to a separate output (y3a) summed on the
    host, so the kernel tail only runs the pair-1 projection. Outputs
    bf16 (summed in f32 on the host).
"""
import sys

sys.path.insert(0, "/opt/trn_rl_repo")

import numpy as np
import ml_dtypes

import concourse.bacc as bacc
import concourse.tile as tile
from concourse import mybir
from concourse.bass_utils import run_bass_kernel_spmd

F32 = mybir.dt.float32
BF16 = mybir.dt.bfloat16
FP16 = mybir.dt.float16
EXP = mybir.ActivationFunctionType.Exp
MULT = mybir.AluOpType.mult
ADD = mybir.AluOpType.add

B, N, DIM = 2, 2048, 1024
H, DH = 16, 64
INNER = H * DH
SCALE = DH ** -0.5
NCORES = 8
HPC = H // (NCORES // B)      # heads per core = 4
NPAIR = HPC // 2              # head pairs per core = 2

P = 128
NT = N // 512                 # 4 n-tiles of 512
DC = DIM // P                 # 8 d-chunks
JTILES = N // P               # 16 j-tiles
JB = JTILES // 2              # 8 j-batches (2 j-tiles each)

PAIRSWAP = [i ^ 1 for i in range(32)]

GP_T2 = False                 # gpsimd cannot read PSUM; rotary stays on DVE

_CACHE = {}

# fill-unit schedule: (nq, pair) -> {jb or "pre": [units]}.  Unit kinds:
#   ("qk", key, t, half)  8 matmuls N=256 of one qkv e-chunk column half
#   ("rot", key, t, half) DVE rotary of that half -> qrot/krot
#   ("v", t, nsub)        8 matmuls N=256 -> v_aug[t] rows nsub
#   ("yp", nq, nsub)      4 matmuls N=512: both-pair y projection rows
#   ("y3a", nsub)         2 matmuls N=512: pair-0 nq=3 partial -> y3a
def _mk_sched():
    qkrot = lambda key, t: [("qk", key, t, 0), ("qk", key, t, 1),
                            ("rot", key, t, 0), ("rot", key, t, 1)]
    vt = lambda t: [("v", t, 0), ("v", t, 1), ("v", t, 2), ("v", t, 3)]
    return {
        (0, 0): {0: vt(0),
                 1: qkrot("k0", 1),
                 2: vt(1),
                 3: qkrot("k0", 2),
                 4: vt(2),
                 5: qkrot("k0", 3),
                 6: vt(3),
                 7: [("qk", "k1", 0, 0), ("qk", "k1", 0, 1),
                     ("rot", "k1", 0, 0), ("rot", "k1", 0, 1),
                     ("qk", "q1", 0, 0), ("qk", "q1", 0, 1)]},
        (0, 1): {"pre": [("rot", "q1", 0, 0), ("rot", "q1", 0, 1)],
                 0: qkrot("k1", 1),
                 1: qkrot("q0", 1),
                 2: qkrot("k1", 2),
                 4: qkrot("k1", 3)},
        (1, 0): {0: qkrot("q1", 1),
                 2: qkrot("q0", 2),
                 4: [("yp", 0, 0)], 5: [("yp", 0, 1)],
                 6: [("yp", 0, 2)], 7: [("yp", 0, 3)]},
        (1, 1): {0: qkrot("q1", 2),
                 2: qkrot("q0", 3)},
        (2, 0): {0: qkrot("q1", 3),
                 4: [("yp", 1, 0)], 6: [("yp", 1, 1)]},
        (2, 1): {1: [("yp", 1, 2)], 3: [("yp", 1, 3)]},
        (3, 0): {1: [("yp", 2, 0)], 3: [("yp", 2, 1)],
                 5: [("yp", 2, 2)], 7: [("yp", 2, 3)]},
        (3, 1): {2: [("y3a", 0)], 3: [("y3a", 1)],
                 4: [("y3a", 2)], 5: [("y3a", 3)]},
    }


def _build():
    nc = bacc.Bacc(None, target_bir_lowering=False, debug=False)
    with tile.TileContext(nc) as tc:
        with tc.tile_pool(name="dram", bufs=1, space="DRAM") as dram, \
             tc.tile_pool(name="const", bufs=1) as const, \
             tc.tile_pool(name="perst", bufs=1) as perst, \
             tc.tile_pool(name="tmp", bufs=1) as tmp, \
             tc.tile_pool(name="ps", bufs=1, space="PSUM") as ps:
            # ---------------- DRAM I/O ----------------
            # wqkP: host-packed [4*128, 1024] bf16, row block ech, cols (c,e)
            # so each partition strip is 2KB contiguous (fast DMA).
            # wvP: host-packed [128, 2048] bf16, cols (c,e).
            xT_d = dram.tile([DIM, N], BF16, kind="ExternalInput", name="xT", uniquify=False)
            wqkP_d = dram.tile([512, DIM], BF16, kind="ExternalInput", name="wqkP", uniquify=False)
            wvP_d = dram.tile([P, 2048], BF16, kind="ExternalInput", name="wvP", uniquify=False)
            cq_d = dram.tile([P, N], BF16, kind="ExternalInput", name="cq", uniquify=False)
            sq_d = dram.tile([P, N], BF16, kind="ExternalInput", name="sq", uniquify=False)
            ck_d = dram.tile([P, N], BF16, kind="ExternalInput", name="ck", uniquify=False)
            sk_d = dram.tile([P, N], BF16, kind="ExternalInput", name="sk", uniquify=False)
            woT_d = dram.tile([256, DIM], BF16, kind="ExternalInput", name="woT", uniquify=False)
            y_d = dram.tile([N, DIM], BF16, kind="ExternalOutput", name="y", uniquify=False)
            y3a_d = dram.tile([512, DIM], BF16, kind="ExternalOutput", name="y3a", uniquify=False)

            xT_r = xT_d.rearrange("(c p) n -> p c n", p=P)
            cs_src = {"cq": cq_d, "sq": sq_d, "ck": ck_d, "sk": sk_d}
            ECH = {"q0": 0, "q1": 1, "k0": 2, "k1": 3}

            # ---------------- SBUF tiles (alloc; DMA ordered below) -----
            wqk = {(e, cp): const.tile([P, 2, P], BF16, name=f"wqk{e}_{cp}")
                   for e in range(4) for cp in range(4)}
            wv = [const.tile([P, 2, 256], BF16, name=f"wv{cp}") for cp in range(4)]
            wo = [const.tile([P, DIM], BF16, name=f"wo{p}") for p in range(NPAIR)]
            cs = {(k, t): const.tile([P, 512], BF16, name=f"{k}{t}")
                  for k in cs_src for t in range(NT)}
            xt = {(t, c): perst.tile([P, 512], BF16, name=f"xt{t}_{c}")
                  for t in range(NT) for c in range(DC)}

            # dma_start issue costs ~600ns of sequencer time; SP and ACT
            # are both HWDGE engines, so alternate them during the
            # bootstrap to halve the serialized issue latency.
            alt = {"i": 0}

            def D(dst, src, boot=False):
                eng = nc.sync
                if boot:
                    eng = (nc.sync, nc.scalar)[alt["i"] % 2]
                    alt["i"] += 1
                eng.dma_start(dst, src)

            def dma_w(ech, cp, boot=False):
                D(wqk[(ech, cp)][:, :, :],
                  wqkP_d[ech * P:(ech + 1) * P, cp * 256:(cp + 1) * 256], boot)

            def dma_x(t, c, boot=False):
                D(xt[(t, c)][:, :], xT_r[:, c, t * 512:(t + 1) * 512], boot)

            def dma_cs(k, t, boot=False):
                D(cs[(k, t)][:, :], cs_src[k][:, t * 512:(t + 1) * 512], boot)

            def dma_cs_half(k, t, ph):
                # partition-split so the two halves land on two queues
                rows = slice(ph * 64, (ph + 1) * 64)
                D(cs[(k, t)][rows, :], cs_src[k][rows, t * 512:(t + 1) * 512],
                  boot=True)

            # ---------------- DMA emission: consumption order -----------
            dma_w(2, 0, boot=True)               # k0 weights c0,c1
            dma_x(0, 0, boot=True)
            for cp in range(1, 4):
                dma_w(2, cp, boot=True)          # k0 weights rest
                dma_x(0, cp, boot=True)
            for c in range(4, DC):
                dma_x(0, c, boot=True)
            for ph in range(2):
                dma_cs_half("ck", 0, ph)
                dma_cs_half("sk", 0, ph)
            for cp in range(4):
                dma_w(0, cp, boot=True)          # q0 weights
            for ph in range(2):
                dma_cs_half("cq", 0, ph)
                dma_cs_half("sq", 0, ph)
            for cp in range(4):
                D(wv[cp][:, :, :], wvP_d[:, cp * 512:(cp + 1) * 512], boot=True)
            for c in range(DC):
                dma_x(1, c, boot=True)
            dma_cs("ck", 1, boot=True)   # k0-tile1 rotary needs these early
            dma_cs("sk", 1, boot=True)
            for cp in range(4):
                dma_w(3, cp)          # k1 weights
                dma_w(1, cp)          # q1 weights
            dma_cs("ck", 2)
            dma_cs("sk", 2)
            for c in range(DC):
                dma_x(2, c)
            dma_cs("ck", 3)
            dma_cs("sk", 3)
            for c in range(DC):
                dma_x(3, c)
            for t in range(1, NT):
                dma_cs("cq", t)
                dma_cs("sq", t)
            for p in range(NPAIR):
                nc.sync.dma_start(wo[p][:, :], woT_d[p * P:(p + 1) * P, :])

            # ---------------- small constants / persistent --------------
            ones_b = const.tile([1, 64], BF16)
            nc.vector.memset(ones_b[:, :], 1.0)

            qrot = [[perst.tile([P, 512], BF16, name=f"qrot{p}_{t}")
                     for t in range(NT)] for p in range(NPAIR)]
            krot = [[perst.tile([P, 512], BF16, name=f"krot{p}_{t}")
                     for t in range(NT)] for p in range(NPAIR)]
            v_aug = [perst.tile([P, 4, HPC, 65], FP16, name=f"vaug{t}")
                     for t in range(NT)]
            for t in range(NT):
                nc.vector.memset(v_aug[t][:, :, :, 64:65], 1.0)
            aoT = [[perst.tile([P, 512], BF16, name=f"aoT{p}_{t}")
                    for t in range(NT)] for p in range(NPAIR)]

            # ---------------- fill units --------------------------------
            pqk_live = {}

            def qk_half(key, t, h):
                # one column half of a qkv e-chunk: psum[:, h*256:+256]
                if h == 0:
                    pq = ps.tile([P, 512], F32, name=f"pqk", tag="m", bufs=2)
                    pqk_live[(key, t)] = pq
                else:
                    pq = pqk_live[(key, t)]
                ech = ECH[key]
                for c in range(DC):
                    nc.tensor.matmul(pq[:, h * 256:(h + 1) * 256],
                                     wqk[(ech, c // 2)][:, c % 2, :],
                                     xt[(t, c)][:, h * 256:(h + 1) * 256],
                                     start=(c == 0), stop=(c == DC - 1))

            def rot_half(key, t, h):
                pq = pqk_live[(key, t)]
                pair = int(key[1])
                dest = (krot if key[0] == "k" else qrot)[pair][t]
                cos = cs[("ck" if key[0] == "k" else "cq", t)]
                sin = cs[("sk" if key[0] == "k" else "sq", t)]
                sl = slice(h * 256, (h + 1) * 256)
                t1 = tmp.tile([P, 256], BF16, name="t1", tag="t1", bufs=3)
                t2 = tmp.tile([P, 256], BF16, name="t2", tag="t2", bufs=3)
                t3 = tmp.tile([P, 256], BF16, name="t3", tag="t3", bufs=3)
                nc.vector.tensor_tensor(t1[:, :], pq[:, sl], cos[:, sl], op=MULT)
                eng = nc.gpsimd if GP_T2 else nc.vector
                eng.tensor_tensor(t2[:, :], pq[:, sl], sin[:, sl], op=MULT)
                nc.vector.stream_shuffle(t3[:, :], t2[:, :], PAIRSWAP)
                nc.vector.tensor_tensor(dest[:, sl], t1[:, :], t3[:, :], op=ADD)

            def v_nsub(t, nsub):
                pv = ps.tile([P, 256], F32, name="pv", tag="m", bufs=2)
                off = nsub * P
                for c in range(DC):
                    nc.tensor.matmul(pv[:, :],
                                     xt[(t, c)][:, off:off + P],
                                     wv[c // 2][:, c % 2, :],
                                     start=(c == 0), stop=(c == DC - 1))
                nc.vector.tensor_copy(
                    v_aug[t][:, nsub, :, 0:64],
                    pv[:, :].rearrange("p (h d) -> p h d", h=HPC))

            def yproj_nsub(nq, nsub):
                ys = tmp.tile([P, DIM], BF16, name="ys", tag="ys", bufs=4)
                nsl = slice(nsub * P, (nsub + 1) * P)
                for dh2 in range(2):
                    py = ps.tile([P, 512], F32, name="py", tag="m", bufs=2)
                    dsl = slice(dh2 * 512, (dh2 + 1) * 512)
                    for pair in range(NPAIR):
                        nc.tensor.matmul(py[:, :],
                                         aoT[pair][nq][:, nsl],
                                         wo[pair][:, dsl],
                                         start=(pair == 0), stop=(pair == NPAIR - 1))
                    nc.vector.tensor_copy(ys[:, dsl], py[:, :])
                r0 = nq * 512 + nsub * P
                nc.sync.dma_start(y_d[r0:r0 + P, :], ys[:, :])

            def yproj_pair_nsub(nq, pair, nsub, out_d, row0, tail=False):
                ys = tmp.tile([P, DIM], BF16, name="ysp", tag="ys", bufs=4)
                nsl = slice(nsub * P, (nsub + 1) * P)
                r0 = row0 + nsub * P
                for dh2 in range(2):
                    py = ps.tile([P, 512], F32, name="pyp", tag="m", bufs=2)
                    dsl = slice(dh2 * 512, (dh2 + 1) * 512)
                    nc.tensor.matmul(py[:, :], aoT[pair][nq][:, nsl],
                                     wo[pair][:, dsl], start=True, stop=True)
                    if tail:
                        # ACT copies + partition-split DMAs (4 queues/nsub)
                        nc.scalar.activation(ys[:, dsl], py[:, :],
                                             mybir.ActivationFunctionType.Copy)
                        nc.sync.dma_start(out_d[r0:r0 + 64, dsl], ys[0:64, dsl])
                        nc.sync.dma_start(out_d[r0 + 64:r0 + P, dsl],
                                          ys[64:P, dsl])
                    else:
                        nc.vector.tensor_copy(ys[:, dsl], py[:, :])
                if not tail:
                    nc.sync.dma_start(out_d[r0:r0 + P, :], ys[:, :])

            def emit_unit(u):
                if u[0] == "qk":
                    qk_half(u[1], u[2], u[3])
                elif u[0] == "rot":
                    rot_half(u[1], u[2], u[3])
                elif u[0] == "v":
                    v_nsub(u[1], u[2])
                elif u[0] == "yp":
                    yproj_nsub(u[1], u[2])
                elif u[0] == "y3a":
                    yproj_pair_nsub(NT - 1, 0, u[1], y3a_d, 0)

            # ---------------- attention ---------------------------------
            def mk_av(pair, jb, exs, pavs):
                def go():
                    for jl in range(2):
                        jt = jb * 2 + jl
                        for h in range(2):
                            nc.tensor.matmul(pavs[h][:, :],
                                             v_aug[jt // 4][:, jt % 4, pair * 2 + h, :],
                                             exs[h][:, jl, :],
                                             start=(jt == 0), stop=(jt == JTILES - 1))
                return go

            COPY_F = mybir.ActivationFunctionType.Copy

            def evac_fused(nq, pair, pavs):
                # final-round evacuation: copies on the now-idle ACT engine,
                # per-head recip chains (partition bases must be 32-aligned),
                # fused broadcast psum + single final copy/multiply.
                av2 = tmp.tile([P, 512], F32, name="av2", tag="av2", bufs=1)
                pbc2 = ps.tile([P, 512], F32, name="pbc2", tag="m", bufs=2)
                for h in range(2):
                    sm_sb = tmp.tile([1, 512], F32, name="sm_f", tag="sms", bufs=4)
                    nc.scalar.activation(sm_sb[:, :], pavs[h][64:65, :], COPY_F)
                    rc = tmp.tile([1, 512], F32, name="rc_f", tag="rc", bufs=2)
                    nc.vector.reciprocal_approx_fast(rc[:, :], sm_sb[:, :])
                    rcr = tmp.tile([1, 512], BF16, name="rcr_f", tag="rcr", bufs=2)
                    nc.vector.tensor_copy(rcr[:, :], rc[:, :])
                    nc.tensor.matmul(pbc2[h * 64:(h + 1) * 64, :],
                                     ones_b[:, :], rcr[:, :],
                                     start=True, stop=True)
                    nc.scalar.activation(av2[h * 64:(h + 1) * 64, :],
                                         pavs[h][0:64, :], COPY_F)
                bc2 = tmp.tile([P, 512], F32, name="bc2", tag="bc2", bufs=1)
                nc.scalar.activation(bc2[:, :], pbc2[:, :], COPY_F)
                nc.vector.tensor_tensor(aoT[pair][nq][:, :],
                                        av2[:, :], bc2[:, :], op=MULT)

            def mk_evac(nq, pair, pavs):
                if nq == NT - 1 and pair == 1:
                    return lambda: evac_fused(nq, pair, pavs)
                mm_bcast = nq == NT - 1
                def go():
                    for h in range(2):
                        av_sb = tmp.tile([64, 512], F32, name="av_sb", tag="avs", bufs=3)
                        sm_sb = tmp.tile([1, 512], F32, name="sm_sb", tag="sms", bufs=4)
                        nc.vector.tensor_copy(av_sb[:, :], pavs[h][0:64, :])
                        nc.vector.tensor_copy(sm_sb[:, :], pavs[h][64:65, :])
                        rc = tmp.tile([1, 512], F32, name="rc", tag="rc", bufs=2)
                        nc.vector.reciprocal_approx_fast(rc[:, :], sm_sb[:, :])
                        bc = tmp.tile([64, 512], F32, name="bc", tag="bc", bufs=2)
                        if mm_bcast:
                            # broadcast via K=1 ones-matmul (no DMA latency)
                            rcr = tmp.tile([1, 512], BF16, name="rcr", tag="rcr", bufs=2)
                            nc.vector.tensor_copy(rcr[:, :], rc[:, :])
                            pbc = ps.tile([64, 512], F32, name="pbc", tag="m", bufs=2)
                            nc.tensor.matmul(pbc[:, :], ones_b[:, :], rcr[:, :],
                                             start=True, stop=True)
                            nc.vector.tensor_copy(bc[:, :], pbc[:, :])
                        else:
                            rd = dram.tile([1, 512], F32, name="rd", tag="rd", bufs=2)
                            nc.sync.dma_start(rd[:, :], rc[:, :])
                            nc.sync.dma_start(bc[:, :], rd.to_broadcast([64, 512]))
                        rows = slice(h * 64, (h + 1) * 64)
                        # all-SBUF operands -> offload to the idle gpsimd
                        nc.gpsimd.tensor_tensor(aoT[pair][nq][rows, :],
                                                av_sb[:, :], bc[:, :], op=MULT)
                return go

            SCHED = _mk_sched()
            pending = []

            def attention_round(nq, pair):
                nonlocal pending
                sched = SCHED.get((nq, pair), {})
                for u in sched.get("pre", []):
                    emit_unit(u)
                pavs = None
                for jb in range(JB):
                    scs = [ps.tile([P, 2, 512], F32, name=f"sc{h}",
                                   tag="s", bufs=2) for h in range(2)]
                    for jl in range(2):
                        jt = jb * 2 + jl
                        kt = krot[pair][jt // 4]
                        jsl = slice((jt % 4) * P, (jt % 4 + 1) * P)
                        for h in range(2):
                            rows = slice(h * 64, (h + 1) * 64)
                            nc.tensor.matmul(scs[h][:, jl, :], kt[rows, jsl],
                                             qrot[pair][nq][rows, :],
                                             start=True, stop=True,
                                             tile_position=(h * 64, 0))
                    exs = {}
                    for h in range(2):
                        e = tmp.tile([P, 2, 512], FP16, name=f"ex{h}",
                                     tag="ex", bufs=6)
                        nc.scalar.activation(e[:, :, :], scs[h][:, :, :], EXP)
                        exs[h] = e
                    for u in sched.get(jb, []):
                        emit_unit(u)
                    for f in pending:
                        f()
                    pending = []
                    if jb == 0:
                        pavs = [ps.tile([65, 512], F32, name=f"pav{h}",
                                        tag="av", bufs=2) for h in range(2)]
                    pending.append(mk_av(pair, jb, exs, pavs))
                pending.append(mk_evac(nq, pair, pavs))

            # ---------------- emission ----------------------------------
            # bootstrap tile 0: interleave the k0/q0 chunk matmuls so each
            # arriving x-chunk enables 4 matmuls (denser PE, earlier rotary),
            # then one full-width rotary per key.
            def rot_boot(key):
                pq = pqk_live[(key, 0)]
                dest = (krot if key[0] == "k" else qrot)[0][0]
                cos = cs[("ck" if key[0] == "k" else "cq", 0)]
                sin = cs[("sk" if key[0] == "k" else "sq", 0)]
                t1 = tmp.tile([P, 512], BF16, name="t1b", tag="t1b", bufs=2)
                t2 = tmp.tile([P, 512], BF16, name="t2b", tag="t2b", bufs=2)
                t3 = tmp.tile([P, 512], BF16, name="t3b", tag="t3b", bufs=2)
                nc.vector.tensor_tensor(t1[:, :], pq[:, :], cos[:, :], op=MULT)
                nc.vector.tensor_tensor(t2[:, :], pq[:, :], sin[:, :], op=MULT)
                nc.vector.stream_shuffle(t3[:, :], t2[:, :], PAIRSWAP)
                nc.vector.tensor_tensor(dest[:, :], t1[:, :], t3[:, :], op=ADD)

            for key in ("k0", "q0"):
                pqk_live[(key, 0)] = ps.tile([P, 512], F32, name=f"pqk_{key}",
                                             tag="m", bufs=2)
            # h0 wave interleaved k0/q0: two concurrent accumulation groups
            # in DIFFERENT psum banks (same-bank interleave corrupts), so
            # each arriving x-chunk feeds two matmuls.
            for c in range(DC):
                for key in ("k0", "q0"):
                    nc.tensor.matmul(pqk_live[(key, 0)][:, 0:256],
                                     wqk[(ECH[key], c // 2)][:, c % 2, :],
                                     xt[(0, c)][:, 0:256],
                                     start=(c == 0), stop=(c == DC - 1))
            for c in range(DC):
                nc.tensor.matmul(pqk_live[("k0", 0)][:, 256:512],
                                 wqk[(2, c // 2)][:, c % 2, :],
                                 xt[(0, c)][:, 256:512],
                                 start=(c == 0), stop=(c == DC - 1))
            rot_boot("k0")           # DVE chain overlaps the q0 h1 matmuls
            for c in range(DC):
                nc.tensor.matmul(pqk_live[("q0", 0)][:, 256:512],
                                 wqk[(0, c // 2)][:, c % 2, :],
                                 xt[(0, c)][:, 256:512],
                                 start=(c == 0), stop=(c == DC - 1))
            rot_boot("q0")

            for nq in range(NT):
                for pair in range(NPAIR):
                    attention_round(nq, pair)

            for f in pending:                    # AV(7) + evac of (3,1)
                f()
            pending = []
            for nsub in range(4):                # tail: pair-1 nq=3 proj
                yproj_pair_nsub(NT - 1, 1, nsub, y_d, (NT - 1) * 512, tail=True)
    nc.compile()
    return nc


def _host_prep(x, rotary_emb, w_qkv, w_out):
    """Build the 8 per-core input maps."""
    bf16 = ml_dtypes.bfloat16
    x = np.asarray(x, dtype=np.float32)
    rotary_emb = np.asarray(rotary_emb, dtype=np.float32)
    w_qkv = np.asarray(w_qkv, dtype=np.float32)
    w_out = np.asarray(w_out, dtype=np.float32)

    # interleaved dh permutation: new row 2i <- dim i, 2i+1 <- dim 32+i
    perm = np.empty(DH, dtype=np.int64)
    perm[0::2] = np.arange(32)
    perm[1::2] = np.arange(32) + 32
    pair_swap = np.arange(DH) ^ 1

    cos = np.cos(rotary_emb).T[perm]                      # [dh, n] permuted
    sin = np.sin(rotary_emb).T[perm]
    sign = np.where(perm < 32, -1.0, 1.0)[:, None].astype(np.float32)
    sin_eff = sign * sin
    sin_pre = sin_eff[pair_swap]                          # pre-swapped
    c2 = np.concatenate([cos, cos], axis=0)               # [128, n]
    s2 = np.concatenate([sin_pre, sin_pre], axis=0)
    cq = np.ascontiguousarray((SCALE * c2).astype(bf16))
    sq = np.ascontiguousarray((SCALE * s2).astype(bf16))
    ck = np.ascontiguousarray(c2.astype(bf16))
    sk = np.ascontiguousarray(s2.astype(bf16))

    in_maps = []
    for core in range(NCORES):
        b = core // (NCORES // B)
        g = core % (NCORES // B)
        heads = range(4 * g, 4 * g + HPC)
        q_rows = np.concatenate([h * DH + perm for h in heads])
        k_rows = np.concatenate([INNER + h * DH + perm for h in heads])
        v_rows = np.arange(2 * INNER + 4 * g * DH, 2 * INNER + (4 * g + HPC) * DH)
        wqkT = w_qkv[np.concatenate([q_rows, k_rows])].T    # [1024, 512]
        # pack per e-chunk with c-major columns: [4*128, 1024] where row
        # block ech, partition p, cols c*128+e = wqkT[c*128+p, ech*128+e]
        wqkP = np.ascontiguousarray(
            wqkT.reshape(DC, P, 4, P).transpose(2, 1, 0, 3)
                .reshape(512, DIM).astype(bf16))
        wvT = w_qkv[v_rows].T                               # [1024, 256]
        wvP = np.ascontiguousarray(
            wvT.reshape(DC, P, 256).transpose(1, 0, 2)
               .reshape(P, 2048).astype(bf16))
        woT = np.ascontiguousarray(
            w_out[:, 4 * g * DH:(4 * g + HPC) * DH].T.astype(bf16))
        xT = np.ascontiguousarray(x[b].T.astype(bf16))
        in_maps.append({
            "xT": xT, "wqkP": wqkP, "wvP": wvP,
            "cq": cq, "sq": sq, "ck": ck, "sk": sk, "woT": woT,
        })
    return in_maps


def kernel(x, rotary_emb, w_qkv, w_out, b_out, _trace=False):
    if "nc" not in _CACHE:
        _CACHE["nc"] = _build()
    nc = _CACHE["nc"]
    in_maps = _host_prep(x, rotary_emb, w_qkv, w_out)
    res = run_bass_kernel_spmd(nc, in_maps, core_ids=list(range(NCORES)),
                               trace=_trace)
    _CACHE["last_result"] = res
    y = np.zeros((B, N, DIM), dtype=np.float32)
    for core in range(NCORES):
        b = core // (NCORES // B)
        y[b] += np.asarray(res.results[core]["y"], dtype=np.float32)
        y[b, (NT - 1) * 512:] += np.asarray(res.results[core]["y3a"],
                                            dtype=np.float32)
    y += np.asarray(b_out, dtype=np.float32)[None, None, :]
    return y


# revision 32
# speedup vs baseline: 1.0764x; 1.0764x over previous
"""Multi-head attention (QKV proj + rotary + softmax attention + out proj)
for Trainium2, sharded over 8 NeuronCores.

Problem: x[2,2048,1024], 16 heads x dh=64, rotary embedding, softmax
attention, output projection + bias.

Sharding: batch x head-group. Core c handles batch c//4 and the 4 heads
[4*(c%4), 4*(c%4)+4). Each core computes its QKV slice, rotary, attention,
and a partial output projection; the host sums the 4 partial projections
per batch and adds the bias.

Device-side design (per core, everything in "transposed" layout, all
matmul operands bf16/fp16 so every matmul streams at 1 col/cycle and
DMA+SBUF traffic is halved):
  - DMAs are emitted fine-grained (64KB-256KB) in consumption order so
    the first matmul can start ~3us in instead of waiting for all
    constants; all four xT n-tiles are persistent in SBUF.
  - qkvT e-chunks = W @ x^T accumulated over 8 d-chunks, in two 256-col
    half-units so fill work can be interleaved at ~850ns granularity.
  - rotary on the fp32 psum via DVE (+ the sin-multiply on the otherwise
    idle gpsimd engine): q*cos + pairswap(q*sin_pre), dh interleaved
    ([0,32,1,33,...]) so rotate_half is an adjacent-lane stream_shuffle.
    Outputs bf16.
  - dots: scoresT[j,n] = krotT-slice @ qrotT, two heads packed in the PE
    array via tile_position row-tiling (K=64 each). Per-(jl,h) psum
    tiles [128,512] with 4 psum banks of buffering so the PE never
    blocks on the exp pipeline.
  - softmax without max-subtraction (logits are O(+-6)): ACT exp per
    [128,512] psum tile, output fp16.
  - AV: lhsT = [v | ones] (M=65, fp16) so row 64 accumulates the softmax
    denominators for free; fp32 psum accumulation over the 16 j-tiles.
    AV emission is software-pipelined one j-batch behind the dots/exp so
    the in-order PE queue never waits on ACT.
  - normalize: reciprocal_approx_fast of the sums row,
    partition-broadcast via a DRAM round-trip DMA (K=1 ones-matmul on
    the last block to keep the tail off the DMA latency), one DVE
    multiply -> aoT (bf16).
  - output proj: y[n,d] accumulated over the two head-pair e-chunks,
    emitted per-128-row units threaded through later rounds; the last
    block's pair-0 partial goes to a separate output (y3a) summed on the
    host, so the kernel tail only runs the pair-1 projection. Outputs
    bf16 (summed in f32 on the host).
"""
import sys

sys.path.insert(0, "/opt/trn_rl_repo")

import numpy as np
import ml_dtypes

import concourse.bacc as bacc
import concourse.tile as tile
from concourse import mybir
from concourse.bass_utils import run_bass_kernel_spmd

F32 = mybir.dt.float32
BF16 = mybir.dt.bfloat16
FP16 = mybir.dt.float16
EXP = mybir.ActivationFunctionType.Exp
MULT = mybir.AluOpType.mult
ADD = mybir.AluOpType.add

B, N, DIM = 2, 2048, 1024
H, DH = 16, 64
INNER = H * DH
SCALE = DH ** -0.5
NCORES = 8
HPC = H // (NCORES // B)      # heads per core = 4
NPAIR = HPC // 2              # head pairs per core = 2

P = 128
NT = N // 512                 # 4 n-tiles of 512
DC = DIM // P                 # 8 d-chunks
JTILES = N // P               # 16 j-tiles
JB = JTILES // 2              # 8 j-batches (2 j-tiles each)

PAIRSWAP = [i ^ 1 for i in range(32)]

GP_T2 = False                 # gpsimd cannot read PSUM; rotary stays on DVE

_CACHE = {}

# fill-unit schedule: (nq, pair) -> {jb or "pre": [units]}.  Unit kinds:
#   ("qk", key, t, half)  8 matmuls N=256 of one qkv e-chunk column half
#   ("rot", key, t, half) DVE rotary of that half -> qrot/krot
#   ("v", t, nsub)        8 matmuls N=256 -> v_aug[t] rows nsub
#   ("yp", nq, nsub)      4 matmuls N=512: both-pair y projection rows
#   ("y3a", nsub)         2 matmuls N=512: pair-0 nq=3 partial -> y3a
def _mk_sched():
    qkrot = lambda key, t: [("qk", key, t, 0), ("qk", key, t, 1),
                            ("rot", key, t, 0), ("rot", key, t, 1)]
    vt = lambda t: [("v", t, 0), ("v", t, 1), ("v", t, 2), ("v", t, 3)]
    return {
        (0, 0): {0: vt(0),
                 1: qkrot("k0", 1),
                 2: vt(1),
                 3: qkrot("k0", 2),
                 4: vt(2),
                 5: qkrot("k0", 3),
                 6: vt(3),
                 7: [("qk", "k1", 0, 0), ("qk", "k1", 0, 1),
                     ("rot", "k1", 0, 0), ("rot", "k1", 0, 1),
                     ("qk", "q1", 0, 0), ("qk", "q1", 0, 1)]},
        (0, 1): {"pre": [("rot", "q1", 0, 0), ("rot", "q1", 0, 1)],
                 0: qkrot("k1", 1),
                 1: qkrot("q0", 1),
                 2: qkrot("k1", 2),
                 4: qkrot("k1", 3)},
        (1, 0): {0: qkrot("q1", 1),
                 2: qkrot("q0", 2),
                 4: [("yp", 0, 0)], 5: [("yp", 0, 1)],
                 6: [("yp", 0, 2)], 7: [("yp", 0, 3)]},
        (1, 1): {0: qkrot("q1", 2),
                 2: qkrot("q0", 3)},
        (2, 0): {0: qkrot("q1", 3),
                 4: [("yp", 1, 0)], 6: [("yp", 1, 1)]},
        (2, 1): {1: [("yp", 1, 2)], 3: [("yp", 1, 3)]},
        (3, 0): {1: [("yp", 2, 0)], 3: [("yp", 2, 1)],
                 5: [("yp", 2, 2)], 7: [("yp", 2, 3)]},
        (3, 1): {2: [("y3a", 0)], 3: [("y3a", 1)],
                 4: [("y3a", 2)], 5: [("y3a", 3)]},
    }


def _build():
    nc = bacc.Bacc(None, target_bir_lowering=False, debug=False)
    with tile.TileContext(nc) as tc:
        with tc.tile_pool(name="dram", bufs=1, space="DRAM") as dram, \
             tc.tile_pool(name="const", bufs=1) as const, \
             tc.tile_pool(name="perst", bufs=1) as perst, \
             tc.tile_pool(name="tmp", bufs=1) as tmp, \
             tc.tile_pool(name="ps", bufs=1, space="PSUM") as ps:
            # ---------------- DRAM I/O ----------------
            # wqkP: host-packed [4*128, 1024] bf16, row block ech, cols (c,e)
            # so each partition strip is 2KB contiguous (fast DMA).
            # wvP: host-packed [128, 2048] bf16, cols (c,e).
            xT_d = dram.tile([DIM, N], BF16, kind="ExternalInput", name="xT", uniquify=False)
            wqkP_d = dram.tile([512, DIM], BF16, kind="ExternalInput", name="wqkP", uniquify=False)
            wvP_d = dram.tile([P, 2048], BF16, kind="ExternalInput", name="wvP", uniquify=False)
            cq_d = dram.tile([P, N], BF16, kind="ExternalInput", name="cq", uniquify=False)
            sq_d = dram.tile([P, N], BF16, kind="ExternalInput", name="sq", uniquify=False)
            ck_d = dram.tile([P, N], BF16, kind="ExternalInput", name="ck", uniquify=False)
            sk_d = dram.tile([P, N], BF16, kind="ExternalInput", name="sk", uniquify=False)
            woT_d = dram.tile([256, DIM], BF16, kind="ExternalInput", name="woT", uniquify=False)
            y_d = dram.tile([N, DIM], BF16, kind="ExternalOutput", name="y", uniquify=False)
            y3a_d = dram.tile([512, DIM], BF16, kind="ExternalOutput", name="y3a", uniquify=False)

            xT_r = xT_d.rearrange("(c p) n -> p c n", p=P)
            cs_src = {"cq": cq_d, "sq": sq_d, "ck": ck_d, "sk": sk_d}
            ECH = {"q0": 0, "q1": 1, "k0": 2, "k1": 3}

            # ---------------- SBUF tiles (alloc; DMA ordered below) -----
            wqk = {(e, cp): const.tile([P, 2, P], BF16, name=f"wqk{e}_{cp}")
                   for e in range(4) for cp in range(4)}
            wv = [const.tile([P, 2, 256], BF16, name=f"wv{cp}") for cp in range(4)]
            wo = [const.tile([P, DIM], BF16, name=f"wo{p}") for p in range(NPAIR)]
            cs = {(k, t): const.tile([P, 512], BF16, name=f"{k}{t}")
                  for k in cs_src for t in range(NT)}
            xt = {(t, c): perst.tile([P, 512], BF16, name=f"xt{t}_{c}")
                  for t in range(NT) for c in range(DC)}

            # dma_start issue costs ~600ns of sequencer time; SP and ACT
            # are both HWDGE engines, so alternate them during the
            # bootstrap to halve the serialized issue latency.
            alt = {"i": 0}

            def D(dst, src, boot=False):
                eng = nc.sync
                if boot:
                    eng = (nc.sync, nc.scalar)[alt["i"] % 2]
                    alt["i"] += 1
                eng.dma_start(dst, src)

            def dma_w(ech, cp, boot=False):
                D(wqk[(ech, cp)][:, :, :],
                  wqkP_d[ech * P:(ech + 1) * P, cp * 256:(cp + 1) * 256], boot)

            def dma_x(t, c, boot=False):
                D(xt[(t, c)][:, :], xT_r[:, c, t * 512:(t + 1) * 512], boot)

            def dma_cs(k, t, boot=False):
                D(cs[(k, t)][:, :], cs_src[k][:, t * 512:(t + 1) * 512], boot)

            def dma_cs_half(k, t, ph):
                # partition-split so the two halves land on two queues
                rows = slice(ph * 64, (ph + 1) * 64)
                D(cs[(k, t)][rows, :], cs_src[k][rows, t * 512:(t + 1) * 512],
                  boot=True)

            # ---------------- DMA emission: consumption order -----------
            dma_w(2, 0, boot=True)               # k0 weights c0,c1
            dma_x(0, 0, boot=True)
            for cp in range(1, 4):
                dma_w(2, cp, boot=True)          # k0 weights rest
                dma_x(0, cp, boot=True)
            for c in range(4, DC):
                dma_x(0, c, boot=True)
            for ph in range(2):
                dma_cs_half("ck", 0, ph)
                dma_cs_half("sk", 0, ph)
            for cp in range(4):
                dma_w(0, cp, boot=True)          # q0 weights
            for ph in range(2):
                dma_cs_half("cq", 0, ph)
                dma_cs_half("sq", 0, ph)
            for cp in range(4):
                D(wv[cp][:, :, :], wvP_d[:, cp * 512:(cp + 1) * 512], boot=True)
            for c in range(DC):
                dma_x(1, c, boot=True)
            dma_cs("ck", 1, boot=True)   # k0-tile1 rotary needs these early
            dma_cs("sk", 1, boot=True)
            for cp in range(4):
                dma_w(3, cp)          # k1 weights
                dma_w(1, cp)          # q1 weights
            dma_cs("ck", 2)
            dma_cs("sk", 2)
            for c in range(DC):
                dma_x(2, c)
            dma_cs("ck", 3)
            dma_cs("sk", 3)
            for c in range(DC):
                dma_x(3, c)
            for t in range(1, NT):
                dma_cs("cq", t)
                dma_cs("sq", t)
            for p in range(NPAIR):
                nc.sync.dma_start(wo[p][:, :], woT_d[p * P:(p + 1) * P, :])

            # ---------------- small constants / persistent --------------
            ones_b = const.tile([1, 64], BF16)
            nc.vector.memset(ones_b[:, :], 1.0)

            qrot = [[perst.tile([P, 512], BF16, name=f"qrot{p}_{t}")
                     for t in range(NT)] for p in range(NPAIR)]
            krot = [[perst.tile([P, 512], BF16, name=f"krot{p}_{t}")
                     for t in range(NT)] for p in range(NPAIR)]
            v_aug = [perst.tile([P, 4, HPC, 65], FP16, name=f"vaug{t}")
                     for t in range(NT)]
            for t in range(NT):
                nc.vector.memset(v_aug[t][:, :, :, 64:65], 1.0)
            aoT = [[perst.tile([P, 512], BF16, name=f"aoT{p}_{t}")
                    for t in range(NT)] for p in range(NPAIR)]

            # ---------------- fill units --------------------------------
            pqk_live = {}

            def qk_half(key, t, h):
                # one column half of a qkv e-chunk: psum[:, h*256:+256]
                if h == 0:
                    pq = ps.tile([P, 512], F32, name=f"pqk", tag="m", bufs=2)
                    pqk_live[(key, t)] = pq
                else:
                    pq = pqk_live[(key, t)]
                ech = ECH[key]
                for c in range(DC):
                    nc.tensor.matmul(pq[:, h * 256:(h + 1) * 256],
                                     wqk[(ech, c // 2)][:, c % 2, :],
                                     xt[(t, c)][:, h * 256:(h + 1) * 256],
                                     start=(c == 0), stop=(c == DC - 1))

            def rot_half(key, t, h):
                pq = pqk_live[(key, t)]
                pair = int(key[1])
                dest = (krot if key[0] == "k" else qrot)[pair][t]
                cos = cs[("ck" if key[0] == "k" else "cq", t)]
                sin = cs[("sk" if key[0] == "k" else "sq", t)]
                sl = slice(h * 256, (h + 1) * 256)
                t1 = tmp.tile([P, 256], BF16, name="t1", tag="t1", bufs=3)
                t2 = tmp.tile([P, 256], BF16, name="t2", tag="t2", bufs=3)
                t3 = tmp.tile([P, 256], BF16, name="t3", tag="t3", bufs=3)
                nc.vector.tensor_tensor(t1[:, :], pq[:, sl], cos[:, sl], op=MULT)
                eng = nc.gpsimd if GP_T2 else nc.vector
                eng.tensor_tensor(t2[:, :], pq[:, sl], sin[:, sl], op=MULT)
                nc.vector.stream_shuffle(t3[:, :], t2[:, :], PAIRSWAP)
                nc.vector.tensor_tensor(dest[:, sl], t1[:, :], t3[:, :], op=ADD)

            def v_nsub(t, nsub):
                pv = ps.tile([P, 256], F32, name="pv", tag="m", bufs=2)
                off = nsub * P
                for c in range(DC):
                    nc.tensor.matmul(pv[:, :],
                                     xt[(t, c)][:, off:off + P],
                                     wv[c // 2][:, c % 2, :],
                                     start=(c == 0), stop=(c == DC - 1))
                nc.vector.tensor_copy(
                    v_aug[t][:, nsub, :, 0:64],
                    pv[:, :].rearrange("p (h d) -> p h d", h=HPC))

            def yproj_nsub(nq, nsub):
                ys = tmp.tile([P, DIM], BF16, name="ys", tag="ys", bufs=4)
                nsl = slice(nsub * P, (nsub + 1) * P)
                for dh2 in range(2):
                    py = ps.tile([P, 512], F32, name="py", tag="m", bufs=2)
                    dsl = slice(dh2 * 512, (dh2 + 1) * 512)
                    for pair in range(NPAIR):
                        nc.tensor.matmul(py[:, :],
                                         aoT[pair][nq][:, nsl],
                                         wo[pair][:, dsl],
                                         start=(pair == 0), stop=(pair == NPAIR - 1))
                    nc.vector.tensor_copy(ys[:, dsl], py[:, :])
                r0 = nq * 512 + nsub * P
                nc.sync.dma_start(y_d[r0:r0 + P, :], ys[:, :])

            def yproj_pair_nsub(nq, pair, nsub, out_d, row0, tail=False):
                ys = tmp.tile([P, DIM], BF16, name="ysp", tag="ys", bufs=4)
                nsl = slice(nsub * P, (nsub + 1) * P)
                r0 = row0 + nsub * P
                for dh2 in range(2):
                    py = ps.tile([P, 512], F32, name="pyp", tag="m", bufs=2)
                    dsl = slice(dh2 * 512, (dh2 + 1) * 512)
                    nc.tensor.matmul(py[:, :], aoT[pair][nq][:, nsl],
                                     wo[pair][:, dsl], start=True, stop=True)
                    if tail:
                        # ACT copies + partition-split DMAs (4 queues/nsub)
                        nc.scalar.activation(ys[:, dsl], py[:, :],
                                             mybir.ActivationFunctionType.Copy)
                        nc.sync.dma_start(out_d[r0:r0 + 64, dsl], ys[0:64, dsl])
                        nc.sync.dma_start(out_d[r0 + 64:r0 + P, dsl],
                                          ys[64:P, dsl])
                    else:
                        nc.vector.tensor_copy(ys[:, dsl], py[:, :])
                if not tail:
                    nc.sync.dma_start(out_d[r0:r0 + P, :], ys[:, :])

            def emit_unit(u):
                if u[0] == "qk":
                    qk_half(u[1], u[2], u[3])
                elif u[0] == "rot":
                    rot_half(u[1], u[2], u[3])
                elif u[0] == "v":
                    v_nsub(u[1], u[2])
                elif u[0] == "yp":
                    yproj_nsub(u[1], u[2])
                elif u[0] == "y3a":
                    yproj_pair_nsub(NT - 1, 0, u[1], y3a_d, 0)

            # ---------------- attention ---------------------------------
            def mk_av(pair, jb, exs, pavs):
                def go():
                    # h-major: stay in one pav psum bank for both j-tiles
                    for h in range(2):
                        for jl in range(2):
                            jt = jb * 2 + jl
                            nc.tensor.matmul(pavs[h][:, :],
                                             v_aug[jt // 4][:, jt % 4, pair * 2 + h, :],
                                             exs[h][:, jl, :],
                                             start=(jt == 0), stop=(jt == JTILES - 1))
                return go

            COPY_F = mybir.ActivationFunctionType.Copy

            def evac_fused(nq, pair, pavs):
                # final-round evacuation: copies on the now-idle ACT engine,
                # per-head recip chains (partition bases must be 32-aligned),
                # fused broadcast psum + single final copy/multiply.
                av2 = tmp.tile([P, 512], F32, name="av2", tag="av2", bufs=1)
                pbc2 = ps.tile([P, 512], F32, name="pbc2", tag="m", bufs=2)
                for h in range(2):
                    sm_sb = tmp.tile([1, 512], F32, name="sm_f", tag="sms", bufs=4)
                    nc.scalar.activation(sm_sb[:, :], pavs[h][64:65, :], COPY_F)
                    rc = tmp.tile([1, 512], F32, name="rc_f", tag="rc", bufs=2)
                    nc.vector.reciprocal_approx_fast(rc[:, :], sm_sb[:, :])
                    rcr = tmp.tile([1, 512], BF16, name="rcr_f", tag="rcr", bufs=2)
                    nc.vector.tensor_copy(rcr[:, :], rc[:, :])
                    nc.tensor.matmul(pbc2[h * 64:(h + 1) * 64, :],
                                     ones_b[:, :], rcr[:, :],
                                     start=True, stop=True)
                    nc.scalar.activation(av2[h * 64:(h + 1) * 64, :],
                                         pavs[h][0:64, :], COPY_F)
                bc2 = tmp.tile([P, 512], F32, name="bc2", tag="bc2", bufs=1)
                nc.scalar.activation(bc2[:, :], pbc2[:, :], COPY_F)
                nc.vector.tensor_tensor(aoT[pair][nq][:, :],
                                        av2[:, :], bc2[:, :], op=MULT)

            def mk_evac(nq, pair, pavs):
                if nq == NT - 1 and pair == 1:
                    return lambda: evac_fused(nq, pair, pavs)
                mm_bcast = nq == NT - 1
                def go():
                    for h in range(2):
                        av_sb = tmp.tile([64, 512], F32, name="av_sb", tag="avs", bufs=3)
                        sm_sb = tmp.tile([1, 512], F32, name="sm_sb", tag="sms", bufs=4)
                        nc.vector.tensor_copy(av_sb[:, :], pavs[h][0:64, :])
                        nc.vector.tensor_copy(sm_sb[:, :], pavs[h][64:65, :])
                        rc = tmp.tile([1, 512], F32, name="rc", tag="rc", bufs=2)
                        nc.vector.reciprocal_approx_fast(rc[:, :], sm_sb[:, :])
                        bc = tmp.tile([64, 512], F32, name="bc", tag="bc", bufs=2)
                        if mm_bcast:
                            # broadcast via K=1 ones-matmul (no DMA latency)
                            rcr = tmp.tile([1, 512], BF16, name="rcr", tag="rcr", bufs=2)
                            nc.vector.tensor_copy(rcr[:, :], rc[:, :])
                            pbc = ps.tile([64, 512], F32, name="pbc", tag="m", bufs=2)
                            nc.tensor.matmul(pbc[:, :], ones_b[:, :], rcr[:, :],
                                             start=True, stop=True)
                            nc.vector.tensor_copy(bc[:, :], pbc[:, :])
                        else:
                            rd = dram.tile([1, 512], F32, name="rd", tag="rd", bufs=2)
                            nc.sync.dma_start(rd[:, :], rc[:, :])
                            nc.sync.dma_start(bc[:, :], rd.to_broadcast([64, 512]))
                        rows = slice(h * 64, (h + 1) * 64)
                        # all-SBUF operands -> offload to the idle gpsimd
                        nc.gpsimd.tensor_tensor(aoT[pair][nq][rows, :],
                                                av_sb[:, :], bc[:, :], op=MULT)
                return go

            SCHED = _mk_sched()
            pending = []

            def attention_round(nq, pair):
                nonlocal pending
                sched = SCHED.get((nq, pair), {})
                for u in sched.get("pre", []):
                    emit_unit(u)
                pavs = None
                for jb in range(JB):
                    scs = [ps.tile([P, 2, 512], F32, name=f"sc{h}",
                                   tag="s", bufs=2) for h in range(2)]
                    # h-major order: consecutive matmuls stay in one psum
                    # bank (bank alternation costs extra per instruction)
                    for h in range(2):
                        rows = slice(h * 64, (h + 1) * 64)
                        for jl in range(2):
                            jt = jb * 2 + jl
                            kt = krot[pair][jt // 4]
                            jsl = slice((jt % 4) * P, (jt % 4 + 1) * P)
                            nc.tensor.matmul(scs[h][:, jl, :], kt[rows, jsl],
                                             qrot[pair][nq][rows, :],
                                             start=True, stop=True,
                                             tile_position=(h * 64, 0))
                    exs = {}
                    for h in range(2):
                        e = tmp.tile([P, 2, 512], FP16, name=f"ex{h}",
                                     tag="ex", bufs=6)
                        nc.scalar.activation(e[:, :, :], scs[h][:, :, :], EXP)
                        exs[h] = e
                    for u in sched.get(jb, []):
                        emit_unit(u)
                    for f in pending:
                        f()
                    pending = []
                    if jb == 0:
                        pavs = [ps.tile([65, 512], F32, name=f"pav{h}",
                                        tag="av", bufs=2) for h in range(2)]
                    pending.append(mk_av(pair, jb, exs, pavs))
                pending.append(mk_evac(nq, pair, pavs))

            # ---------------- emission ----------------------------------
            # bootstrap tile 0: interleave the k0/q0 chunk matmuls so each
            # arriving x-chunk enables 4 matmuls (denser PE, earlier rotary),
            # then one full-width rotary per key.
            def rot_boot(key):
                pq = pqk_live[(key, 0)]
                dest = (krot if key[0] == "k" else qrot)[0][0]
                cos = cs[("ck" if key[0] == "k" else "cq", 0)]
                sin = cs[("sk" if key[0] == "k" else "sq", 0)]
                t1 = tmp.tile([P, 512], BF16, name="t1b", tag="t1b", bufs=2)
                t2 = tmp.tile([P, 512], BF16, name="t2b", tag="t2b", bufs=2)
                t3 = tmp.tile([P, 512], BF16, name="t3b", tag="t3b", bufs=2)
                nc.vector.tensor_tensor(t1[:, :], pq[:, :], cos[:, :], op=MULT)
                nc.vector.tensor_tensor(t2[:, :], pq[:, :], sin[:, :], op=MULT)
                nc.vector.stream_shuffle(t3[:, :], t2[:, :], PAIRSWAP)
                nc.vector.tensor_tensor(dest[:, :], t1[:, :], t3[:, :], op=ADD)

            for key in ("k0", "q0"):
                qk_half(key, 0, 0)
                qk_half(key, 0, 1)
                rot_boot(key)        # DVE chain overlaps the next PE wave

            for nq in range(NT):
                for pair in range(NPAIR):
                    attention_round(nq, pair)

            for f in pending:                    # AV(7) + evac of (3,1)
                f()
            pending = []
            for nsub in range(4):                # tail: pair-1 nq=3 proj
                yproj_pair_nsub(NT - 1, 1, nsub, y_d, (NT - 1) * 512, tail=True)
    nc.compile()
    return nc


def _host_prep(x, rotary_emb, w_qkv, w_out):
    """Build the 8 per-core input maps."""
    bf16 = ml_dtypes.bfloat16
    x = np.asarray(x, dtype=np.float32)
    rotary_emb = np.asarray(rotary_emb, dtype=np.float32)
    w_qkv = np.asarray(w_qkv, dtype=np.float32)
    w_out = np.asarray(w_out, dtype=np.float32)

    # interleaved dh permutation: new row 2i <- dim i, 2i+1 <- dim 32+i
    perm = np.empty(DH, dtype=np.int64)
    perm[0::2] = np.arange(32)
    perm[1::2] = np.arange(32) + 32
    pair_swap = np.arange(DH) ^ 1

    cos = np.cos(rotary_emb).T[perm]                      # [dh, n] permuted
    sin = np.sin(rotary_emb).T[perm]
    sign = np.where(perm < 32, -1.0, 1.0)[:, None].astype(np.float32)
    sin_eff = sign * sin
    sin_pre = sin_eff[pair_swap]                          # pre-swapped
    c2 = np.concatenate([cos, cos], axis=0)               # [128, n]
    s2 = np.concatenate([sin_pre, sin_pre], axis=0)
    cq = np.ascontiguousarray((SCALE * c2).astype(bf16))
    sq = np.ascontiguousarray((SCALE * s2).astype(bf16))
    ck = np.ascontiguousarray(c2.astype(bf16))
    sk = np.ascontiguousarray(s2.astype(bf16))

    in_maps = []
    for core in range(NCORES):
        b = core // (NCORES // B)
        g = core % (NCORES // B)
        heads = range(4 * g, 4 * g + HPC)
        q_rows = np.concatenate([h * DH + perm for h in heads])
        k_rows = np.concatenate([INNER + h * DH + perm for h in heads])
        v_rows = np.arange(2 * INNER + 4 * g * DH, 2 * INNER + (4 * g + HPC) * DH)
        wqkT = w_qkv[np.concatenate([q_rows, k_rows])].T    # [1024, 512]
        # pack per e-chunk with c-major columns: [4*128, 1024] where row
        # block ech, partition p, cols c*128+e = wqkT[c*128+p, ech*128+e]
        wqkP = np.ascontiguousarray(
            wqkT.reshape(DC, P, 4, P).transpose(2, 1, 0, 3)
                .reshape(512, DIM).astype(bf16))
        wvT = w_qkv[v_rows].T                               # [1024, 256]
        wvP = np.ascontiguousarray(
            wvT.reshape(DC, P, 256).transpose(1, 0, 2)
               .reshape(P, 2048).astype(bf16))
        woT = np.ascontiguousarray(
            w_out[:, 4 * g * DH:(4 * g + HPC) * DH].T.astype(bf16))
        xT = np.ascontiguousarray(x[b].T.astype(bf16))
        in_maps.append({
            "xT": xT, "wqkP": wqkP, "wvP": wvP,
            "cq": cq, "sq": sq, "ck": ck, "sk": sk, "woT": woT,
        })
    return in_maps


def kernel(x, rotary_emb, w_qkv, w_out, b_out, _trace=False):
    if "nc" not in _CACHE:
        _CACHE["nc"] = _build()
    nc = _CACHE["nc"]
    in_maps = _host_prep(x, rotary_emb, w_qkv, w_out)
    res = run_bass_kernel_spmd(nc, in_maps, core_ids=list(range(NCORES)),
                               trace=_trace)
    _CACHE["last_result"] = res
    y = np.zeros((B, N, DIM), dtype=np.float32)
    for core in range(NCORES):
        b = core // (NCORES // B)
        y[b] += np.asarray(res.results[core]["y"], dtype=np.float32)
        y[b, (NT - 1) * 512:] += np.asarray(res.results[core]["y3a"],
                                            dtype=np.float32)
    y += np.asarray(b_out, dtype=np.float32)[None, None, :]
    return y
